# revision 13
# baseline (speedup 1.0000x reference)
"""Trainium2 Bass kernel for nn_AffineLayer (topk_masking):
out[b, f] = max_p(x[b] . ww[f, p]) * scale[f] + bias[f]

Shapes (hardcoded per problem spec):
  x     (2048, 1, 8, 8)  -> xf (2048, 64)
  ww    (1024, 64, 1, 8, 8) -> wwf (1024, 64, 64)   (f, p, i)
  scale (1, 1024), bias (1, 1024)
  out   (2048, 1024)

Sharding: f tensor-parallel over 8 cores (F_SH = 128 per core), x replicated.

Per-core layout (b on partitions, x stationary):
  stationary = xT[:, 128m:128(m+1)] : (i=64, b=128)   one load per b-tile
  moving     = wt[:, p-range, :]    : (i=64, 4p, 128f) = 512 cols
  psum token = (b=128, 16p, 128f) fp32 -- 4 matmuls, 4 banks

Hardware rule (NCC_IBVF027): an instruction reads at most ONE non-scalar
input from PSUM, so PSUM egress is capped at 1 elem/lane/cycle on each of
DVE and ACT. The drain bundles the whole 16:1 token reduction with egress:
  'D' DVE tensor_reduce(max) over the token's 16 planes, PSUM -> one fp16
      slot (BT, 128f) in a single instruction (egress + reduce, no tail)
  'V' ACT copies token -> fp16 SBUF; DVE tensor_reduce 16:1
  'G' ACT copies token -> fp16 SBUF; GPSIMD TT-max chain 16->8->4->2->1
Slots land in a per-group tile (BT, 4bt, 4tok, 128f) fp16; a 2-instr DVE
combine produces (BT, 4bt, 128f) fp32, one DMA per group to y (2048, 128).
"""

import os
import sys

if "/opt/trn_rl_repo" not in sys.path:
    sys.path.insert(0, "/opt/trn_rl_repo")

import numpy as np

import concourse.bass as bass
import concourse.mybir as mybir
from concourse.tile import TileContext
from concourse.bass_utils import run_bass_kernel_spmd

# Problem dims (hardcoded)
B, FDIM, P, IDIM = 2048, 1024, 64, 64
N_CORES = 8
F_SH = FDIM // N_CORES  # 128
BT = 128  # b-tile (partition dim)
NBT = B // BT  # 16 b-tiles
GRP = 4  # b-tiles per group
NG = NBT // GRP  # 4 groups
TPB = 4  # tokens per b-tile
PPT = P // TPB  # p-planes per token = 16
MMP = 4  # p-planes per matmul (512 moving cols)

# ---- Tunables ----------------------------------------------------------
# Per-group token pattern, 16 chars (b-tile-major):
#  D = DVE 16:1 reduce from PSUM
#  V = ACT stage -> DVE 16:1 reduce (dominated; kept for experiments)
#  G = ACT stage -> GPSIMD TT chain 16->8->4->2->1
#  H = ACT stage -> GPSIMD TT 16->8 -> DVE 8:1 reduce
TOK = os.environ.get("KTOK", "DGVDGDVGDGDVGDGD")
MM_DT_NAME = os.environ.get("KMM_DT", "bfloat16")
STAGE_DT_NAME = os.environ.get("KSTAGE_DT", "float16")
REPS = int(os.environ.get("KREPS", "0"))
NWCH = int(os.environ.get("KNWCH", "4"))  # wt load chunks
COMBINE_GPS = os.environ.get("KCOMB_GPS", "0") == "1"  # c1 combine on GPSIMD
# ------------------------------------------------------------------------

F32 = mybir.dt.float32
STAGE_DT = getattr(mybir.dt, STAGE_DT_NAME)
MM_DT = getattr(mybir.dt, MM_DT_NAME)
MX = mybir.AluOpType.max


def split_multiwaits(nc):
    """This walrus build allows at most ONE sem wait per instruction.
    Tile's wait assignment can emit several; hoist extras onto inserted
    sequencer nops immediately before the over-subscribed instruction
    (same engine, program order preserved => identical semantics)."""
    wid = 0
    for f in nc.m.functions:
        for bb in f.blocks:
            il = bb.instructions
            i = 0
            while i < len(il):
                ins = il[i]
                si = getattr(ins, "sync_info", None)
                if si is not None and si.on_wait and len(si.on_wait) > 1:
                    waits = list(si.on_wait)
                    si.on_wait = waits[-1:]
                    carriers = []
                    for w in waits[:-1]:
                        wid += 1
                        carriers.append(
                            mybir.InstNoOp(
                                name=f"WSPLIT-{wid}",
                                engine=ins.engine,
                                sync_info=mybir.SyncInfo(on_wait=[w], on_update=[]),
                            )
                        )
                    il[i:i] = carriers
                    i += len(carriers)
                i += 1


def build_nc(tok=None, fixup=True, affine=False):
    tok = tok or TOK
    pats = tok.split(";")
    if len(pats) == 1:
        pats = pats * NG
    assert len(pats) == NG
    for p_ in pats:
        assert len(p_) == GRP * TPB and set(p_) <= set("DVGH"), p_

    nc = bass.Bass()
    xt_d = nc.dram_tensor("xt", [IDIM, B], MM_DT, kind="ExternalInput")
    wt_d = nc.dram_tensor("wt", [IDIM, P, F_SH], MM_DT, kind="ExternalInput")
    if affine:
        sc_d = nc.dram_tensor("scale4", [BT, GRP, F_SH], F32, kind="ExternalInput")
        bi_d = nc.dram_tensor("bias4", [BT, GRP, F_SH], F32, kind="ExternalInput")
    y_d = nc.dram_tensor("y", [B, F_SH], F32, kind="ExternalOutput")

    PW = P // NWCH  # planes per wt chunk

    with TileContext(nc) as tc:
        with (
            tc.tile_pool(name="const", bufs=1) as const,
            tc.tile_pool(name="psum", bufs=2, space="PSUM") as psum,
            tc.tile_pool(name="stv", bufs=4) as stv,
            tc.tile_pool(name="gtmp", bufs=4) as gtmp,
            tc.tile_pool(name="slots", bufs=2) as slotsp,
            tc.tile_pool(name="c1p", bufs=2) as c1p,
            tc.tile_pool(name="outs", bufs=2) as outs,
        ):
            xt = const.tile([IDIM, B], MM_DT)
            nc.sync.dma_start(out=xt[:], in_=xt_d[:])
            wchunks = [
                const.tile([IDIM, PW, F_SH], MM_DT, name=f"wt{c}") for c in range(NWCH)
            ]
            for c in range(NWCH):
                nc.sync.dma_start(
                    out=wchunks[c][:], in_=wt_d[:, c * PW : (c + 1) * PW, :]
                )
            if affine:
                sc = const.tile([BT, GRP, F_SH], F32)
                nc.sync.dma_start(out=sc[:], in_=sc_d[:])
                bi = const.tile([BT, GRP, F_SH], F32)
                nc.sync.dma_start(out=bi[:], in_=bi_d[:])
            warm = const.tile([BT, 2], F32)
            nc.vector.memset(warm[:], 0.0)
            nc.scalar.activation(
                out=warm[:, 1:2], in_=warm[:, 0:1],
                func=mybir.ActivationFunctionType.Copy,
            )

            import contextlib

            loop_cm = (
                tc.For_i(0, REPS, 1, hint_engines=(mybir.EngineType.PE,))
                if REPS > 0
                else contextlib.nullcontext()
            )
            with loop_cm:
                for g in range(NG):
                    pat = pats[g]
                    slots = slotsp.tile([BT, GRP, TPB, F_SH], STAGE_DT, tag="sl")
                    for m in range(GRP):
                        bt = g * GRP + m
                        stat = xt[:, bt * BT : (bt + 1) * BT]
                        for t in range(TPB):
                            kind = pat[m * TPB + t]
                            pt = psum.tile([BT, PPT, F_SH], F32, tag="ps")
                            for q in range(TPB):
                                p0 = t * PPT + q * MMP
                                nc.tensor.matmul(
                                    pt[:, q * MMP : (q + 1) * MMP, :],
                                    stat,
                                    wchunks[p0 // PW][:, p0 % PW : p0 % PW + MMP, :],
                                    start=True,
                                    stop=True,
                                )
                            dst = slots[:, m, t, :]
                            if kind == "D":
                                # 16:1 max-reduce straight from PSUM: view the
                                # token as (BT, f, p) and reduce innermost p
                                nc.vector.tensor_reduce(
                                    dst,
                                    pt[:].rearrange("b p f -> b f p"),
                                    axis=mybir.AxisListType.X,
                                    op=MX,
                                )
                            else:
                                sv = stv.tile([BT, PPT, F_SH], STAGE_DT, tag="sv")
                                nc.scalar.activation(
                                    out=sv[:], in_=pt[:],
                                    func=mybir.ActivationFunctionType.Copy,
                                )
                                if kind == "V":
                                    nc.vector.tensor_reduce(
                                        dst,
                                        sv[:].rearrange("b p f -> b f p"),
                                        axis=mybir.AxisListType.X,
                                        op=MX,
                                    )
                                elif kind == "H":  # GPS 16->8, DVE 8:1 reduce
                                    gt = gtmp.tile(
                                        [BT, PPT // 2, F_SH], STAGE_DT, tag="gt"
                                    )
                                    nc.gpsimd.tensor_max(
                                        gt[:],
                                        sv[:, 0:PPT:2, :],
                                        sv[:, 1:PPT:2, :],
                                    )
                                    nc.vector.tensor_reduce(
                                        dst,
                                        gt[:].rearrange("b p f -> b f p"),
                                        axis=mybir.AxisListType.X,
                                        op=MX,
                                    )
                                else:  # G: GPSIMD TT-max chain 16->8->4->2->1
                                    gt = gtmp.tile(
                                        [BT, PPT // 2, F_SH], STAGE_DT, tag="gt"
                                    )
                                    nc.gpsimd.tensor_max(
                                        gt[:, 0:8, :],
                                        sv[:, 0:PPT:2, :],
                                        sv[:, 1:PPT:2, :],
                                    )
                                    nc.gpsimd.tensor_max(
                                        gt[:, 0:4, :], gt[:, 0:8:2, :], gt[:, 1:8:2, :]
                                    )
                                    nc.gpsimd.tensor_max(
                                        gt[:, 0:2, :], gt[:, 0:4:2, :], gt[:, 1:4:2, :]
                                    )
                                    nc.gpsimd.tensor_max(
                                        dst, gt[:, 0, :], gt[:, 1, :]
                                    )

                    # ---- combine: 4 token-slots -> 1 per b-tile ----------
                    c1 = c1p.tile([BT, GRP, 2, F_SH], STAGE_DT, tag="c1")
                    ceng = nc.gpsimd if COMBINE_GPS else nc.vector
                    ceng.tensor_max(
                        c1[:], slots[:, :, 0:TPB:2, :], slots[:, :, 1:TPB:2, :]
                    )
                    outt = outs.tile([BT, GRP, F_SH], F32, tag="outt")
                    nc.vector.tensor_max(outt[:], c1[:, :, 0, :], c1[:, :, 1, :])
                    if affine:
                        nc.vector.tensor_mul(outt[:], outt[:], sc[:])
                        nc.vector.tensor_add(outt[:], outt[:], bi[:])
                    yv = y_d[g * GRP * BT : (g + 1) * GRP * BT, :].rearrange(
                        "(m b) f -> b m f", m=GRP
                    )
                    nc.sync.dma_start(out=yv, in_=outt[:])

    if fixup:
        split_multiwaits(nc)
    return nc


_CACHED_NC = None


def _get_nc():
    global _CACHED_NC
    if _CACHED_NC is None:
        _CACHED_NC = build_nc()
    return _CACHED_NC


def _to_mm_np(a):
    import ml_dtypes

    np_dt = {"bfloat16": ml_dtypes.bfloat16, "float16": np.float16,
             "float32": np.float32, "float32r": np.float32}[MM_DT_NAME]
    return np.ascontiguousarray(a.astype(np_dt))


def make_in_maps(x, ww, scale, bias, affine=False):
    x = np.asarray(x)
    ww = np.asarray(ww)
    scale = np.asarray(scale)
    bias = np.asarray(bias)

    xf = _to_mm_np(x.reshape(B, IDIM).T.astype(np.float32))  # (64, 2048)
    wwf = ww.reshape(FDIM, P, IDIM)
    sc = scale.reshape(FDIM).astype(np.float32)
    bi = bias.reshape(FDIM).astype(np.float32)

    in_maps = []
    for k in range(N_CORES):
        wk = wwf[k * F_SH : (k + 1) * F_SH]  # (128, 64, 64) = (f, p, i)
        wt = _to_mm_np(wk.transpose(2, 1, 0).astype(np.float32))  # (i, p, f)
        m = {"xt": xf, "wt": wt}
        if affine:
            sck = sc[k * F_SH : (k + 1) * F_SH]
            bik = bi[k * F_SH : (k + 1) * F_SH]
            m["scale4"] = np.ascontiguousarray(
                np.broadcast_to(sck[None, None, :], (BT, GRP, F_SH)).astype(np.float32)
            )
            m["bias4"] = np.ascontiguousarray(
                np.broadcast_to(bik[None, None, :], (BT, GRP, F_SH)).astype(np.float32)
            )
        in_maps.append(m)
    return in_maps


def kernel(x, ww, scale, bias):
    trivial_affine = bool(
        np.all(np.asarray(scale) == 1.0) and np.all(np.asarray(bias) == 0.0)
    )
    affine = not trivial_affine
    in_maps = make_in_maps(x, ww, scale, bias, affine=affine)
    nc = build_nc(affine=affine)
    res = run_bass_kernel_spmd(nc, in_maps, list(range(N_CORES)))
    out = np.empty((B, FDIM), dtype=np.float32)
    for k in range(N_CORES):
        out[:, k * F_SH : (k + 1) * F_SH] = res.results[k]["y"]
    return out
